# revision 8
# baseline (speedup 1.0000x reference)
import sys

sys.path.insert(0, "/opt/trn_rl_repo")

import numpy as np

S, I, E, P = 128, 256, 64, 128
C_ROW, C_GLOB, C_OPM = 8, 8, 32
H_MSA, H_TRI, C_TRI = 8, 4, 128
TS = 4
N_CORES = 8


# ---------------- host-side numpy ops ----------------
def _ln(x, p):
    mu = x.mean(-1, keepdims=True)
    var = ((x - mu) ** 2).mean(-1, keepdims=True)
    return (x - mu) / np.sqrt(var + 1e-5) * p["g"] + p["b"]


def _lin(x, p):
    y = x @ p["w"]
    if "b" in p:
        y = y + p["b"]
    return y


def _sig(x):
    return 1.0 / (1.0 + np.exp(-x))


def _softmax(x, axis):
    m = x.max(axis=axis, keepdims=True)
    e = np.exp(x - m)
    return e / e.sum(axis=axis, keepdims=True)


def _row_attn(msa, pair, p):
    m = _ln(msa, p["ln_m"])
    z = _ln(pair, p["ln_p"])
    b = _lin(z, p["bias"])
    qkv = _lin(m, p["qkv"]).reshape(S, I, 3, H_MSA, C_ROW)
    q, k, v = qkv[:, :, 0], qkv[:, :, 1], qkv[:, :, 2]
    logits = np.einsum("sihc,sjhc->shij", q, k, optimize=True) * (1.0 / np.sqrt(C_ROW))
    logits = logits + b.transpose(2, 0, 1)[None]
    a = _softmax(logits, -1)
    o = np.einsum("shij,sjhc->sihc", a, v, optimize=True)
    g = _sig(_lin(m, p["gate"])).reshape(S, I, H_MSA, C_ROW)
    return _lin((g * o).reshape(S, I, H_MSA * C_ROW), p["out"])


def _glob_attn(msa, p):
    m = _ln(msa, p["ln"])
    qkv = _lin(m, p["qkv"])
    q = qkv[..., : H_MSA * C_GLOB].reshape(S, I, C_GLOB, H_MSA).mean(axis=0)
    k = qkv[..., H_MSA * C_GLOB : (H_MSA + 1) * C_GLOB]
    v = qkv[..., (H_MSA + 1) * C_GLOB :]
    g = _sig(_lin(m, p["gate"])).reshape(S, I, C_GLOB, H_MSA)
    a = _softmax(
        np.einsum("idh,tid->tih", q, k, optimize=True) * (1.0 / np.sqrt(C_GLOB)), 0
    )
    o = np.einsum("tih,tid->idh", a, v, optimize=True)
    return _lin((g * o[None]).reshape(S, I, C_GLOB * H_MSA), p["out"])


def _transition(x, p):
    return _lin(np.maximum(_lin(_ln(x, p["ln"]), p["w1"]), 0.0), p["w2"])


def _opm(msa, p):
    m = _ln(msa, p["ln"])
    a = _lin(m, p["a"])
    b = _lin(m, p["b"])
    o = np.einsum("sic,sjd->ijcd", a, b, optimize=True) / S
    return _lin(o.reshape(I, I, C_OPM * C_OPM), p["out"])


def _tri_mul(pair, p, eq):
    z = _ln(pair, p["ln_in"])
    a = _sig(_lin(z, p["ga"])) * _lin(z, p["pa"])
    b = _sig(_lin(z, p["gb"])) * _lin(z, p["pb"])
    g = _sig(_lin(z, p["g"]))
    o = np.einsum(eq, a, b, optimize=True)
    return g * _lin(_ln(o, p["ln_out"]), p["out"])


def _tri_attn(pair, p, start):
    z = _ln(pair, p["ln"])
    q = _lin(z, p["q"]).reshape(I, I, H_TRI, C_TRI)
    k = _lin(z, p["k"]).reshape(I, I, H_TRI, C_TRI)
    v = _lin(z, p["v"]).reshape(I, I, H_TRI, C_TRI)
    b = _lin(z, p["bias"])
    inv = 1.0 / np.sqrt(C_TRI)
    if start:
        a = _softmax(
            np.einsum("ijhc,ikhc->hijk", q, k, optimize=True) * inv
            + b.transpose(2, 0, 1)[:, None],
            -1,
        )
        o = np.einsum("hijk,ikhc->ijhc", a, v, optimize=True)
    else:
        a = _softmax(
            np.einsum("ijhc,kjhc->hijk", q, k, optimize=True) * inv
            + b.transpose(2, 1, 0)[:, :, None, :],
            -1,
        )
        o = np.einsum("hijk,kjhc->ijhc", a, v, optimize=True)
    g = _sig(_lin(z, p["gate"])).reshape(I, I, H_TRI, C_TRI)
    return _lin((g * o).reshape(I, I, H_TRI * C_TRI), p["out"])


# ---------------- device (bass) pair-transition ----------------
_CACHE = {}


def _build_tri_kernel():
    """Triangle-attention (starting-node form), row-sharded: 32 pair-rows/core.

    Inputs (per core): z rows [8192,128] (pre-layernormed), exp-bias table
    [128,2,4,256], weights. Output: gated attention update rows [8192,128]
    (no out-bias, no residual — host adds those).
    """
    import concourse.bass as bass
    import concourse.tile as tile
    from concourse import mybir, bacc

    HC = H_TRI * C_TRI  # 512
    n_loc = (I // N_CORES) * I
    nc = bacc.Bacc("TRN2", target_bir_lowering=False, debug=False, num_devices=N_CORES)
    z = nc.dram_tensor("z", [n_loc, P], mybir.dt.float32, kind="ExternalInput")
    expb = nc.dram_tensor("expb", [P, 2, H_TRI, I], mybir.dt.float32, kind="ExternalInput")
    wq = nc.dram_tensor("wq", [P, HC], mybir.dt.float32, kind="ExternalInput")
    wk = nc.dram_tensor("wk", [P, HC], mybir.dt.float32, kind="ExternalInput")
    wv = nc.dram_tensor("wv", [P, HC], mybir.dt.float32, kind="ExternalInput")
    wg = nc.dram_tensor("wg", [P, HC], mybir.dt.float32, kind="ExternalInput")
    bg = nc.dram_tensor("bg", [P, HC], mybir.dt.float32, kind="ExternalInput")
    wo = nc.dram_tensor("wo", [P, H_TRI, P], mybir.dt.float32, kind="ExternalInput")
    ident = nc.dram_tensor("ident", [P, P], mybir.dt.float32, kind="ExternalInput")
    upd = nc.dram_tensor("upd", [n_loc, P], mybir.dt.float32, kind="ExternalOutput")

    inv = float(1.0 / np.sqrt(C_TRI))
    n_i = n_loc // I  # 32 rows per core
    f32 = mybir.dt.float32

    with tile.TileContext(nc) as tc:
        with tc.tile_pool(name="singles", bufs=1) as singles, \
             tc.tile_pool(name="work", bufs=2) as work, \
             tc.tile_pool(name="stats", bufs=4) as stats, \
             tc.tile_pool(name="psum", bufs=6, space="PSUM") as psum:
            wq_sb = singles.tile([P, HC], f32); nc.sync.dma_start(wq_sb[:], wq[:])
            wk_sb = singles.tile([P, HC], f32); nc.sync.dma_start(wk_sb[:], wk[:])
            wv_sb = singles.tile([P, HC], f32); nc.sync.dma_start(wv_sb[:], wv[:])
            wg_sb = singles.tile([P, HC], f32); nc.sync.dma_start(wg_sb[:], wg[:])
            bg_sb = singles.tile([P, HC], f32); nc.sync.dma_start(bg_sb[:], bg[:])
            wo_sb = singles.tile([P, H_TRI, P], f32); nc.sync.dma_start(wo_sb[:], wo[:])
            eb_sb = singles.tile([P, 2, H_TRI, I], f32); nc.sync.dma_start(eb_sb[:], expb[:])
            id_sb = singles.tile([P, P], f32); nc.sync.dma_start(id_sb[:], ident[:])

            def pe_t(src_ap, dst_ap, tag):
                ps = psum.tile([P, 512], f32, tag="ps", name=f"t_{tag}")
                nc.tensor.transpose(ps[:, :P], src_ap, id_sb[:])
                nc.scalar.copy(out=dst_ap, in_=ps[:, :P])

            for i in range(n_i):
                # ---- phase 1: per-row linears ----
                zi = work.tile([P, 2, P], f32, tag="zi")
                nc.sync.dma_start(zi[:], z[i * I : (i + 1) * I, :].rearrange("(b p) c -> p b c", p=P))
                zt = work.tile([P, I], f32, tag="zt")
                for b in range(2):
                    pe_t(zi[:, b, :], zt[:, b * P : (b + 1) * P], "z")
                qt = work.tile([P, H_TRI, I], f32, tag="qt")
                kt = work.tile([P, H_TRI, I], f32, tag="kt")
                for h in range(H_TRI):
                    for wsb, dst in ((wq_sb, qt), (wk_sb, kt)):
                        ps = psum.tile([P, 512], f32, tag="ps")
                        nc.tensor.matmul(ps[:, :I], wsb[:, h * P : (h + 1) * P], zt[:], start=True, stop=True)
                        nc.scalar.copy(out=dst[:, h, :], in_=ps[:, :I])
                vi = work.tile([P, 2, HC], f32, tag="vi")
                gi = work.tile([P, 2, HC], f32, tag="gi")
                for b in range(2):
                    ps = psum.tile([P, 512], f32, tag="ps")
                    nc.tensor.matmul(ps[:], zt[:, b * P : (b + 1) * P], wv_sb[:], start=True, stop=True)
                    nc.scalar.copy(out=vi[:, b, :], in_=ps[:])
                    ps2 = psum.tile([P, 512], f32, tag="ps")
                    nc.tensor.matmul(ps2[:], zt[:, b * P : (b + 1) * P], wg_sb[:], start=True, stop=True)
                    nc.vector.tensor_add(out=gi[:, b, :], in0=ps2[:], in1=bg_sb[:])
                    nc.scalar.activation(out=gi[:, b, :], in_=gi[:, b, :],
                                         func=mybir.ActivationFunctionType.Sigmoid)
                og = work.tile([P, 2, HC], f32, tag="og")
                # ---- phase 2: attention per head ----
                for h in range(H_TRI):
                    a_sb = work.tile([P, 2, I], f32, tag="a")
                    s_sb = stats.tile([P, 2], f32, tag="s")
                    for b in range(2):
                        ps = psum.tile([P, 512], f32, tag="ps")
                        nc.tensor.matmul(ps[:, :I], qt[:, h, b * P : (b + 1) * P], kt[:, h, :], start=True, stop=True)
                        nc.scalar.activation(out=a_sb[:, b, :], in_=ps[:, :I],
                                             func=mybir.ActivationFunctionType.Exp, scale=inv)
                        nc.vector.tensor_mul(out=a_sb[:, b, :], in0=a_sb[:, b, :], in1=eb_sb[:, b, h, :])
                        nc.vector.reduce_sum(out=s_sb[:, b : b + 1], in_=a_sb[:, b, :], axis=mybir.AxisListType.X)
                    nc.vector.reciprocal(out=s_sb[:], in_=s_sb[:])
                    at = work.tile([P, 2, I], f32, tag="at")  # [k, kb, j]
                    for b in range(2):
                        for kb in range(2):
                            pe_t(a_sb[:, b, kb * P : (kb + 1) * P], at[:, kb, b * P : (b + 1) * P], "a")
                    for b in range(2):
                        ps = psum.tile([P, 512], f32, tag="ps")
                        for kb in range(2):
                            nc.tensor.matmul(ps[:, :P], at[:, kb, b * P : (b + 1) * P],
                                             vi[:, kb, h * P : (h + 1) * P],
                                             start=(kb == 0), stop=(kb == 1))
                        osc = work.tile([P, P], f32, tag="osc")
                        nc.vector.tensor_scalar_mul(out=osc[:], in0=ps[:, :P], scalar1=s_sb[:, b : b + 1])
                        nc.vector.tensor_mul(out=og[:, b, h * P : (h + 1) * P], in0=osc[:],
                                             in1=gi[:, b, h * P : (h + 1) * P])
                # ---- phase 3: output projection ----
                ogt = work.tile([P, H_TRI, I], f32, tag="ogt")
                for b in range(2):
                    for cb in range(H_TRI):
                        pe_t(og[:, b, cb * P : (cb + 1) * P], ogt[:, cb, b * P : (b + 1) * P], "og")
                for b in range(2):
                    ps = psum.tile([P, 512], f32, tag="ps")
                    for cb in range(H_TRI):
                        nc.tensor.matmul(ps[:, :P], ogt[:, cb, b * P : (b + 1) * P], wo_sb[:, cb, :],
                                         start=(cb == 0), stop=(cb == H_TRI - 1))
                    y = work.tile([P, P], f32, tag="y")
                    nc.vector.tensor_copy(out=y[:], in_=ps[:, :P])
                    nc.sync.dma_start(upd[i * I + b * P : i * I + (b + 1) * P, :], y[:])
    nc.compile()
    return nc


def _tri_attn_device(pair, p, start):
    """Triangle attention via the device kernel. For end-node, run the
    starting-node algorithm on pair^T and transpose the update back."""
    from concourse import bass_utils

    if "tri" not in _CACHE:
        _CACHE["tri"] = _build_tri_kernel()
    nc = _CACHE["tri"]
    x = pair if start else np.ascontiguousarray(pair.transpose(1, 0, 2))
    z = _ln(x, p["ln"])
    btbl = z @ p["bias"]["w"]  # [j,k,h]
    # expb[jj, jb, h, k] = exp(btbl[jb*128+jj, k, h])
    expb = np.ascontiguousarray(
        np.exp(btbl).reshape(2, P, I, H_TRI).transpose(1, 0, 3, 2)
    )
    rows = I // N_CORES
    wo = np.ascontiguousarray(p["out"]["w"], np.float32).reshape(H_TRI, P, P)
    wo = np.ascontiguousarray(wo.transpose(1, 0, 2))  # [P, H, P] -> [c within blk, blk, out]
    base = {
        "expb": expb.astype(np.float32),
        "wq": np.ascontiguousarray(p["q"]["w"], np.float32),
        "wk": np.ascontiguousarray(p["k"]["w"], np.float32),
        "wv": np.ascontiguousarray(p["v"]["w"], np.float32),
        "wg": np.ascontiguousarray(p["gate"]["w"], np.float32),
        "bg": np.broadcast_to(np.asarray(p["gate"]["b"], np.float32), (P, H_TRI * C_TRI)).copy(),
        "wo": wo,
        "ident": np.eye(P, dtype=np.float32),
    }
    in_maps = []
    for c in range(N_CORES):
        m = dict(base)
        m["z"] = np.ascontiguousarray(z[c * rows : (c + 1) * rows].reshape(rows * I, P), np.float32)
        in_maps.append(m)
    res = bass_utils.run_bass_kernel_spmd(nc, in_maps, core_ids=list(range(N_CORES)))
    out = np.concatenate(
        [res.results[c]["upd"].reshape(rows, I, P) for c in range(N_CORES)], axis=0
    )
    out = out + np.asarray(p["out"]["b"], np.float32)
    if not start:
        out = np.ascontiguousarray(out.transpose(1, 0, 2))
    return out


def _build_ptr_kernel():
    import concourse.bass as bass
    import concourse.tile as tile
    from concourse import mybir, bacc

    n_loc = (I // N_CORES) * I  # 8192 positions per core
    nc = bacc.Bacc("TRN2", target_bir_lowering=False, debug=False, num_devices=N_CORES)
    pz = nc.dram_tensor("pz", [n_loc, P], mybir.dt.float32, kind="ExternalInput")
    w1 = nc.dram_tensor("w1", [P, TS * P], mybir.dt.float32, kind="ExternalInput")
    b1 = nc.dram_tensor("b1", [P, TS], mybir.dt.float32, kind="ExternalInput")
    w2 = nc.dram_tensor("w2", [TS, P, P], mybir.dt.float32, kind="ExternalInput")
    b2 = nc.dram_tensor("b2", [P, P], mybir.dt.float32, kind="ExternalInput")
    gam = nc.dram_tensor("gam", [P, P], mybir.dt.float32, kind="ExternalInput")
    bet = nc.dram_tensor("bet", [P, P], mybir.dt.float32, kind="ExternalInput")
    ident = nc.dram_tensor("ident", [P, P], mybir.dt.float32, kind="ExternalInput")
    out = nc.dram_tensor("out", [n_loc, P], mybir.dt.float32, kind="ExternalOutput")

    n_tiles = n_loc // P
    with tile.TileContext(nc) as tc:
        with tc.tile_pool(name="singles", bufs=1) as singles, \
             tc.tile_pool(name="work", bufs=3) as work, \
             tc.tile_pool(name="stats", bufs=4) as stats, \
             tc.tile_pool(name="psum", bufs=2, space="PSUM") as psum:
            w1_sb = singles.tile([P, TS * P], mybir.dt.float32)
            nc.sync.dma_start(w1_sb[:], w1[:])
            b1_sb = singles.tile([P, TS], mybir.dt.float32)
            nc.sync.dma_start(b1_sb[:], b1[:])
            w2_sb = singles.tile([P, TS, P], mybir.dt.float32)
            nc.sync.dma_start(w2_sb[:], w2[:].rearrange("t a b -> a t b"))
            b2_sb = singles.tile([P, P], mybir.dt.float32)
            nc.sync.dma_start(b2_sb[:], b2[:])
            gam_sb = singles.tile([P, P], mybir.dt.float32)
            nc.sync.dma_start(gam_sb[:], gam[:])
            bet_sb = singles.tile([P, P], mybir.dt.float32)
            nc.sync.dma_start(bet_sb[:], bet[:])
            id_sb = singles.tile([P, P], mybir.dt.float32)
            nc.sync.dma_start(id_sb[:], ident[:])
            eps_sb = singles.tile([P, 1], mybir.dt.float32)
            nc.vector.memset(eps_sb, 1e-5)

            for t in range(n_tiles):
                x = work.tile([P, P], mybir.dt.float32, tag="x")
                nc.sync.dma_start(x[:], pz[t * P : (t + 1) * P, :])
                st = stats.tile([P, 6], mybir.dt.float32, tag="st")
                nc.vector.bn_stats(out=st[:], in_=x[:])
                mv = stats.tile([P, 2], mybir.dt.float32, tag="mv")
                nc.vector.bn_aggr(out=mv[:], in_=st[:])
                rstd = stats.tile([P, 1], mybir.dt.float32, tag="rstd")
                nc.scalar.activation(
                    out=rstd[:], in_=mv[:, 1:2],
                    func=mybir.ActivationFunctionType.Sqrt,
                    bias=eps_sb[:], scale=1.0,
                )
                nc.vector.reciprocal(out=rstd[:], in_=rstd[:])
                z = work.tile([P, P], mybir.dt.float32, tag="z")
                nc.vector.tensor_scalar(
                    out=z[:], in0=x[:], scalar1=mv[:, 0:1], scalar2=rstd[:],
                    op0=mybir.AluOpType.subtract, op1=mybir.AluOpType.mult,
                )
                nc.vector.tensor_mul(out=z[:], in0=z[:], in1=gam_sb[:])
                nc.vector.tensor_add(out=z[:], in0=z[:], in1=bet_sb[:])
                zt_ps = psum.tile([P, P], mybir.dt.float32, tag="zt")
                nc.tensor.transpose(zt_ps[:], z[:], id_sb[:])
                zt = work.tile([P, P], mybir.dt.float32, tag="zts")
                nc.scalar.copy(out=zt[:], in_=zt_ps[:])
                ht = work.tile([P, TS, P], mybir.dt.float32, tag="ht")
                for blk in range(TS):
                    h_ps = psum.tile([P, P], mybir.dt.float32, tag="h")
                    nc.tensor.matmul(
                        h_ps[:], w1_sb[:, blk * P : (blk + 1) * P], zt[:],
                        start=True, stop=True,
                    )
                    nc.scalar.activation(
                        out=ht[:, blk, :], in_=h_ps[:],
                        func=mybir.ActivationFunctionType.Relu,
                        bias=b1_sb[:, blk : blk + 1], scale=1.0,
                    )
                o_ps = psum.tile([P, P], mybir.dt.float32, tag="o")
                for blk in range(TS):
                    nc.tensor.matmul(
                        o_ps[:], ht[:, blk, :], w2_sb[:, blk, :],
                        start=(blk == 0), stop=(blk == TS - 1),
                    )
                y = work.tile([P, P], mybir.dt.float32, tag="y")
                nc.vector.tensor_add(out=y[:], in0=o_ps[:], in1=b2_sb[:])
                nc.vector.tensor_add(out=y[:], in0=y[:], in1=x[:])
                nc.sync.dma_start(out[t * P : (t + 1) * P, :], y[:])
    nc.compile()
    return nc


def _ptr_device(pair, p):
    """pair + transition(pair) on 8 NeuronCores, row-sharded."""
    from concourse import bass_utils

    if "ptr" not in _CACHE:
        _CACHE["ptr"] = _build_ptr_kernel()
    nc = _CACHE["ptr"]
    w1 = np.ascontiguousarray(p["w1"]["w"], np.float32)
    b1 = np.ascontiguousarray(p["w1"]["b"], np.float32).reshape(TS, P).T.copy()
    w2 = np.ascontiguousarray(p["w2"]["w"], np.float32).reshape(TS, P, P)
    b2 = np.broadcast_to(np.asarray(p["w2"]["b"], np.float32), (P, P)).copy()
    gam = np.broadcast_to(np.asarray(p["ln"]["g"], np.float32), (P, P)).copy()
    bet = np.broadcast_to(np.asarray(p["ln"]["b"], np.float32), (P, P)).copy()
    ident = np.eye(P, dtype=np.float32)
    rows = I // N_CORES
    in_maps = []
    for c in range(N_CORES):
        shard = np.ascontiguousarray(
            pair[c * rows : (c + 1) * rows].reshape(rows * I, P), np.float32
        )
        in_maps.append({
            "pz": shard, "w1": w1, "b1": b1, "w2": w2, "b2": b2,
            "gam": gam, "bet": bet, "ident": ident,
        })
    res = bass_utils.run_bass_kernel_spmd(nc, in_maps, core_ids=list(range(N_CORES)))
    outs = [res.results[c]["out"].reshape(rows, I, P) for c in range(N_CORES)]
    return np.concatenate(outs, axis=0)


def kernel(extra_msa_rep, pair_rep, params):
    msa = np.asarray(extra_msa_rep, np.float32)
    pair = np.asarray(pair_rep, np.float32)
    params = {
        k: {
            k2: {k3: np.asarray(v3, np.float32) for k3, v3 in v2.items()}
            if isinstance(v2, dict) else np.asarray(v2, np.float32)
            for k2, v2 in v.items()
        }
        for k, v in params.items()
    }
    msa = msa + _row_attn(msa, pair, params["row"])
    msa = msa + _glob_attn(msa, params["glob"])
    msa = msa + _transition(msa, params["mtr"])
    pair = pair + _opm(msa, params["opm"])
    pair = pair + _tri_mul(pair, params["tmo"], "ikc,jkc->ijc")
    pair = pair + _tri_mul(pair, params["tmi"], "kic,kjc->ijc")
    pair = pair + _tri_attn_device(pair, params["tas"], True)
    pair = pair + _tri_attn_device(pair, params["tae"], False)
    pair = _ptr_device(pair, params["ptr"])
    return msa, pair
